# revision 79
# baseline (speedup 1.0000x reference)
"""Trainium2 Bass kernel for IntMultiPrecConv2d (moe_routing).

Math reduction: the two routing masks (argmax one-hot over 2 classes) are
complementary, so the module is exactly

    out[b, c] = scale[c] * conv2d(x, weight)[b, c] + bias[c]

with per-channel scale/bias computed on the host from the routing and the
int-quant parameters.

Device: 3x3 pad-1 conv as shifted matmuls accumulating in PSUM (Cin=128 on
the PE contraction dim, Cout=256 as two 128-wide tiles). All 9 taps run as
5 fp8 DoubleRow matmuls (2 taps packed per PE cell -> 0.5 cycles/row):
  - rows are padded to WP=64 so vertically-adjacent taps pair with stride
    64 bytes (DoubleRow needs pair stride % 16 == 0): (0,3), (1,4), (2,5)
  - taps (6,7) pair through a +1-column-shifted replica of the image built
    in SBUF by the DVE/GpSimd engines (which are otherwise mostly idle)
  - tap 8 pairs against a zero block (zero weight in the second slot)
Eviction: per-channel scale+bias (out = scale*psum + bias -> bf16),
alternating between ScalarE (Identity activation) and DVE (tensor_scalar)
per two-chunk PSUM block so neither engine gates the PE. bf16 output (the
output is bias-dominated, which leaves bf16 rounding at ~1e-3 relative)
is converted to fp32 on the host.

Schedule: all four images' x DMAs are prefetched and the first image's x
and the weights arrive as split DMAs so the PE starts as early as
possible; replica copies and the rescale-param loads ride on the idle
DVE/GpSimd queues (SP's ~650ns-per-issue DGE serialization is the scarce
resource early on); the final half's evictions/DMAs are split fine-grained
across engines to shorten the pipeline tail.

Sharding: data-parallel over batch, 8 cores x 4 images.

Cost-model timeline (TimelineSim, per core): first matmul ~4.1us (DGE
latency + first transfers), PE busy 26.4us with no mid-stream stalls,
tail ~4.4us (last evict + DMA issue + transfer + sem + drain barriers)
=> ~35us total vs the 70.4us baseline.
"""

import numpy as np
import ml_dtypes

B, CIN, COUT, H, W = 32, 128, 256, 56, 56
NCORES = 8
BPC = B // NCORES          # images per core
WP = 64                    # padded row pitch (so vertical tap stride %16==0)
HP = H + 2                 # padded height 58
IMG = HP * WP              # 3712 padded image elems per channel
ZB = 3714                  # zero block base (== 2 mod 16, for the tap-8 pair)
XLEN = 4160                # DMA'd bytes per partition: image + zero slack
XSPLIT = 2752              # first image arrives as [0:XSPLIT) + [XSPLIT:XLEN)
DREP = 4160                # replica base (== 0 mod 16)
RLEN = 56 * WP             # replica length: rows 2..57, col-shifted by +1
XTOT = DREP + RLEN         # 7744 total x-tile width
ROWS = 7                   # output rows per PSUM chunk
NCHUNK = H // ROWS         # 8 chunks per (image, half)
CH = ROWS * W              # 392 output pixels per chunk
BLK = 2                    # chunks per PSUM tile (2 banks)
NBLK = NCHUNK // BLK       # 4 blocks per (image, half)
OUTN = H * W               # 3136

# DoubleRow tap pairs (first_tap, second_tap); tap k = 3*kh + kw reads the
# padded image at offset (row+kh)*WP + kw. Pair rhs stride must be %16==0:
#   (0,3),(1,4),(2,5): vertical pairs, stride WP=64
#   (8,-1): second element reads the zero block at ZB (weights are zero)
#   (6,7): second element reads the +1-shifted replica at DREP
PAIRS = [(0, 3), (1, 4), (2, 5), (8, -1), (6, 7)]
WTW = 512 * len(PAIRS)     # weight tile width

_CACHE = {}


def _build_bass():
    import concourse.bass as bass
    import concourse.tile as tile
    import concourse.mybir as mybir
    from concourse import bacc

    f8 = mybir.dt.float8e4
    f32 = mybir.dt.float32
    bf16 = mybir.dt.bfloat16
    AF = mybir.ActivationFunctionType
    ALU = mybir.AluOpType
    DR = mybir.MatmulPerfMode.DoubleRow

    def mk_ap(proto, steps_counts):
        # Hand-built access pattern (same tensor/offset/partition-pitch as
        # proto): needed for the DoubleRow pair dim (stride spans the
        # replica/zero regions) and the multi-bank eviction reads.
        return bass.AP(proto.tensor, proto.offset,
                       [list(proto.ap[0])] + [list(p) for p in steps_counts])

    nc = bacc.Bacc("TRN2", target_bir_lowering=False, debug=False,
                   num_devices=NCORES)
    xp = nc.dram_tensor("xp", (BPC, CIN, XLEN), f8, kind="ExternalInput").ap()
    wt = nc.dram_tensor("wt", (CIN, WTW), f8, kind="ExternalInput").ap()
    sc = nc.dram_tensor("scale", (2, CIN, 1), f32, kind="ExternalInput").ap()
    bi = nc.dram_tensor("bias", (2, CIN, 1), f32, kind="ExternalInput").ap()
    out = nc.dram_tensor("out", (BPC, 2, CIN, OUTN), bf16,
                         kind="ExternalOutput").ap()

    with tile.TileContext(nc) as tc:
        with (
            tc.tile_pool(name="wpool", bufs=1) as wpool,
            tc.tile_pool(name="bpool", bufs=1) as bpool,
            tc.tile_pool(name="spool", bufs=1) as spool,
            tc.tile_pool(name="xpool", bufs=BPC) as xpool,
            tc.tile_pool(name="opool", bufs=3) as opool,
            tc.tile_pool(name="pspool", bufs=4, space="PSUM") as pspool,
        ):
            # Dummy activation: hoists the 1.3us activation-table load off
            # the first real eviction's critical path. Dummy matmul: starts
            # the PE clock-ramp timer (full clock needs 3us from the first
            # PE instruction) long before the real matmuls begin.
            scr = spool.tile([128, 136], bf16)
            nc.gpsimd.memset(scr[:], 0.0)
            nc.scalar.activation(scr[:, 128:132], scr[:, 132:136],
                                 AF.Identity, bias=0.0, scale=1.0)
            wps = pspool.tile([128, 512 * BLK], f32, tag="ps")
            nc.tensor.matmul(wps[:, 0:8], scr[:, 0:128], scr[:, 0:8],
                             start=True, stop=True)

            # DMA emission order = the serialized-transfer order: image-0
            # rows 0..42, weights (split so each pair sweep's lhsT lands
            # just in time), image-0 tail, then images 1..3 (prefetched, in
            # halves, so the PE never waits at image boundaries).
            xts = [xpool.tile([128, XTOT], f8, name=f"xt{b}")
                   for b in range(BPC)]
            wtile = wpool.tile([128, WTW], f8)
            nc.sync.dma_start(xts[0][:, :XSPLIT], xp[0, :, :XSPLIT])
            nc.sync.dma_start(wtile[:, :512], wt[:, :512])
            nc.sync.dma_start(wtile[:, 512:2048], wt[:, 512:2048])
            nc.sync.dma_start(xts[0][:, XSPLIT:XLEN], xp[0, :, XSPLIT:])
            nc.sync.dma_start(wtile[:, 2048:], wt[:, 2048:])
            for b in range(1, BPC):
                nc.sync.dma_start(xts[b][:, :2080], xp[b, :, :2080])
                nc.sync.dma_start(xts[b][:, 2080:XLEN], xp[b, :, 2080:])
            # rescale params go through the idle GpSimd SWDGE queue: SP's
            # serialized ~650ns/issue backlog would otherwise delay the
            # image 1..3 transfers (and with them the replica copies)
            btile = bpool.tile([128, 4], f32)
            for half in range(2):
                nc.gpsimd.dma_start(btile[:, half:half + 1], bi[half])
                nc.gpsimd.dma_start(btile[:, 2 + half:3 + half], sc[half])
            # +1-column replica of image 0 (for the (6,7) pair): SBUF->SBUF
            # on DVE (2x copy rate), off the DMA path. Two pieces so the
            # first blocks' (6,7) matmuls (whose access-pattern interval
            # only spans piece 0) can start before the full copy lands.
            # Images 2/3 go to the slower-but-idle GpSimd; image 1's
            # replica is emitted after image 0's DVE evictions (DVE is
            # in-order) further below.
            HRL = RLEN // 2
            nc.vector.tensor_copy(xts[0][:, DREP:DREP + HRL],
                                  xts[0][:, 129:129 + HRL])
            nc.vector.tensor_copy(xts[0][:, DREP + HRL:DREP + RLEN],
                                  xts[0][:, 129 + HRL:129 + RLEN])
            for b in (2, 3):
                nc.gpsimd.tensor_copy(xts[b][:, DREP:DREP + RLEN],
                                      xts[b][:, 129:129 + RLEN])

            def matmul_rows(ps, xt, half, grow, psoff, rows, mi):
                k1, k2 = PAIRS[mi]
                kh, kw = divmod(k1, 3)
                o = (grow + kh) * WP + kw
                if mi == 4:
                    d = DREP - 128
                elif k2 >= 0:
                    d = WP
                else:
                    d = ZB - o
                rhs = mk_ap(xt[:, o:o + 1], [[d, 2], [WP, rows], [1, W]])
                lhsT = mk_ap(
                    wtile[:, 512 * mi + 128 * half:512 * mi + 128 * half + 1],
                    [[256, 2], [1, 128]])
                nc.tensor.matmul(ps[:, psoff:psoff + rows * W], lhsT, rhs,
                                 start=(mi == 0), stop=(mi == 4), perf_mode=DR)

            def matmul_chunk(ps, xt, half, j, jj, mi):
                matmul_rows(ps, xt, half, ROWS * j, 512 * jj, ROWS, mi)

            def evict(eng, ps, oimg, half, j0, jj0, nch):
                # bf16(scale * psum + bias) for nch chunks of block ps
                src = mk_ap(ps[:, 512 * jj0:512 * jj0 + 1],
                            [[512, nch], [1, CH]])
                dst = oimg[:, (j0 + jj0) * CH:(j0 + jj0 + nch) * CH]
                evict_ap(eng, src, dst, half)

            def evict_ap(eng, src, dst, half):
                if eng == "act":
                    nc.scalar.activation(dst, src, AF.Identity,
                                         bias=btile[:, half:half + 1],
                                         scale=btile[:, 2 + half:3 + half])
                else:
                    e = nc.vector if eng == "dve" else nc.gpsimd
                    e.tensor_scalar(dst, src,
                                    btile[:, 2 + half:3 + half],
                                    btile[:, half:half + 1],
                                    op0=ALU.mult, op1=ALU.add)

            ib = 0
            for b in range(BPC):
                xt = xts[b]
                for half in range(2):
                    oimg = opool.tile([128, OUTN], bf16)
                    final = b == BPC - 1 and half == 1
                    first = b == 0 and half == 0
                    if first:
                        # Image-0 half-0: phased emission matched to the
                        # staggered arrival of x/weights/replica. Pair-mi
                        # sweeps over chunks 0..3 track the split weight
                        # DMAs; the zero-pair (mi 3) needs the x tail; the
                        # replica pair (mi 4) follows the replica pieces.
                        # Blocks 0/1 finish (and evict) early so their PSUM
                        # tiles are long recycled when half 1 needs them.
                        pss = [pspool.tile([128, 512 * BLK], f32, tag="ps",
                                           name=f"ps0{k}") for k in range(2)]
                        for mi in range(3):
                            for j in range(4):
                                matmul_chunk(pss[j // BLK], xt, half, j,
                                             j % BLK, mi)
                        for j in range(4):
                            matmul_chunk(pss[j // BLK], xt, half, j,
                                         j % BLK, 3)
                        for j in range(4):
                            matmul_chunk(pss[j // BLK], xt, half, j,
                                         j % BLK, 4)
                            if j % BLK == BLK - 1:
                                bk = j // BLK
                                evict("act" if bk % 2 == 0 else "dve",
                                      pss[bk], oimg, half, BLK * bk, 0, BLK)
                        # image-1 replica goes here in the in-order DVE
                        # queue: after block 1's eviction (so it doesn't
                        # delay it) but early enough to beat image 1's
                        # replica-pair matmuls
                        nc.vector.tensor_copy(xts[1][:, DREP:DREP + RLEN],
                                              xts[1][:, 129:129 + RLEN])
                        for bk in (2, 3):
                            ps = pspool.tile([128, 512 * BLK], f32,
                                             tag="ps")
                            for mi in range(5):
                                for jj in range(BLK):
                                    matmul_chunk(ps, xt, half,
                                                 BLK * bk + jj, jj, mi)
                            evict("act" if bk % 2 == 0 else "dve", ps,
                                  oimg, half, BLK * bk, 0, BLK)
                        nc.sync.dma_start(out[b, half, :, :2 * CH],
                                          oimg[:, :2 * CH])
                        nc.sync.dma_start(out[b, half, :, 2 * CH:4 * CH],
                                          oimg[:, 2 * CH:4 * CH])
                        nc.sync.dma_start(out[b, half, :, 4 * CH:6 * CH],
                                          oimg[:, 4 * CH:6 * CH])
                        nc.sync.dma_start(out[b, half, :, 6 * CH:],
                                          oimg[:, 6 * CH:])
                        ib += 4
                        continue
                    for bk in range(NBLK):
                        ps = pspool.tile([128, 512 * BLK], f32, tag="ps")
                        if final and bk == NBLK - 1:
                            # Pipeline tail: chunk 6 completes and drains
                            # first; the last chunk runs as 5-row + 2-row
                            # PSUM groups with evictions spread across
                            # ScalarE/DVE (GpSimd cannot read PSUM) so only
                            # a tiny piece remains after the final matmul.
                            for mi in range(5):
                                matmul_chunk(ps, xt, half, 6, 0, mi)
                            evict("dve", ps, oimg, half, 6, 0, 1)
                            nc.sync.dma_start(
                                out[b, half, :, 4 * CH:7 * CH],
                                oimg[:, 4 * CH:7 * CH])
                            # fresh tile: a new group on the same tile
                            # would WAR-wait on chunk 6's eviction
                            ps7 = pspool.tile([128, 512 * BLK], f32,
                                              tag="ps")
                            for mi in range(5):
                                matmul_rows(ps7, xt, half, ROWS * 7, 0,
                                            ROWS, mi)
                            evict_ap("act", ps7[:, :CH],
                                     oimg[:, 7 * CH:8 * CH], half)
                            nc.sync.dma_start(
                                out[b, half, :, 7 * CH:8 * CH],
                                oimg[:, 7 * CH:8 * CH])
                            ib += 1
                            continue
                        for mi in range(5):
                            for jj in range(BLK):
                                matmul_chunk(ps, xt, half, BLK * bk + jj,
                                             jj, mi)
                        evict("act" if ib % 2 == 0 else "dve", ps, oimg,
                              half, BLK * bk, 0, BLK)
                        if final and bk < 2:
                            lo = BLK * bk * CH
                            nc.sync.dma_start(
                                out[b, half, :, lo:lo + BLK * CH],
                                oimg[:, lo:lo + BLK * CH])
                        elif not final and bk % 2 == 1:
                            lo = BLK * (bk - 1) * CH
                            nc.sync.dma_start(
                                out[b, half, :, lo:lo + 2 * BLK * CH],
                                oimg[:, lo:lo + 2 * BLK * CH])
                        ib += 1
    nc.compile()
    return nc


def _prep(x, weight, alpha_weight, alpha2, b8_2, nb_2, nsh_2, alpha8, b16_8,
          nsh_8):
    """Host-side: routing -> per-channel scale/bias; pack fp8 weights in
    DoubleRow pair layout; zero-pad + fp8-cast x."""
    f64 = np.float64
    sel = np.argmax(np.asarray(alpha_weight), axis=0)
    sw0 = sel == 0
    scale = np.where(sw0,
                     np.asarray(alpha2, f64) * np.exp2(-np.asarray(nsh_2, f64)),
                     np.asarray(alpha8, f64) * np.exp2(-np.asarray(nsh_8, f64)))
    bias = np.where(
        sw0,
        np.asarray(b8_2, f64) * np.exp2(np.asarray(nb_2, f64) -
                                        np.asarray(nsh_2, f64)),
        np.asarray(alpha8, f64) * np.asarray(b16_8, f64) *
        np.exp2(-np.asarray(nsh_8, f64)))

    # wT[ci, k, co] = weight[co, ci, kh, kw], unscaled (fp8 dynamic range)
    wT = np.ascontiguousarray(
        np.asarray(weight, np.float32).transpose(1, 2, 3, 0).reshape(
            CIN, 9, COUT))
    wpk = np.zeros((CIN, WTW), np.float32)
    for p, (k1, k2) in enumerate(PAIRS):
        wpk[:, 512 * p:512 * p + 256] = wT[:, k1]
        if k2 >= 0:
            wpk[:, 512 * p + 256:512 * p + 512] = wT[:, k2]
    wpk = wpk.astype(ml_dtypes.float8_e4m3)

    xpad = np.zeros((B, CIN, XLEN), dtype=ml_dtypes.float8_e4m3)
    xv = xpad[:, :, :IMG].reshape(B, CIN, HP, WP)
    xv[:, :, 1:H + 1, 1:W + 1] = np.asarray(x)

    sc2 = np.ascontiguousarray(scale.astype(np.float32).reshape(2, 128, 1))
    bias2 = np.ascontiguousarray(bias.astype(np.float32).reshape(2, 128, 1))
    return xpad, wpk, sc2, bias2


def _run(inputs, trace=False, **spmd_kwargs):
    from concourse import bass_utils

    if "nc" not in _CACHE:
        _CACHE["nc"] = _build_bass()
    nc = _CACHE["nc"]

    xpad, wpk, sc2, bias2 = _prep(**inputs)
    in_maps = [
        {"xp": xpad[c * BPC:(c + 1) * BPC], "wt": wpk, "scale": sc2,
         "bias": bias2}
        for c in range(NCORES)
    ]
    res = bass_utils.run_bass_kernel_spmd(
        nc, in_maps, core_ids=list(range(NCORES)), trace=trace, **spmd_kwargs)
    parts = [r["out"].reshape(BPC, COUT, H, W) for r in res.results]
    return np.concatenate(parts, axis=0).astype(np.float32), res


def kernel(**inputs) -> np.ndarray:
    out, _ = _run(inputs, trace=False)
    return out
